# revision 13
# baseline (speedup 1.0000x reference)
"""JointNet (RNN-T joint network) Bass kernel for 8 Trainium2 NeuronCores.

Math:  h = tanh(enc @ w1[:640] [:,None,:] + dec @ w1[640:] [None,:,:] + b1)
       out = h @ w2 + b2      over the (B, T, U) grid.

Sharding: data-parallel over T (sequence parallel). Each of the 8 cores gets a
T-slice of 32 (128 (b,t) rows). dec and all weights are replicated.

v2.1 (bf16, trace-tuned): all matmuls bf16 (fp32 PSUM; rel-err ~3.3e-3 vs the
2e-2 gate). Trace findings from v2 applied:
  - host pre-arranges every input into its SBUF layout ([128, ...] row-major)
    so each input is ONE contiguous-per-partition DMA; inputs spread over
    3 queues (sync/gpsimd/vector) with b1/enc-half first so projections
    start ~3.5us in.
  - epb/dp stored bf16 -> grid broadcast-adds hit the DVE 2x 16-bit path.
  - vocab matmul uses N=1024 moving operands: one LDWEIGHTS per (mi,k)
    instead of two (LDW was 690x99ns in the v2 trace).
  - copyback = single DVE tensor_tensor (psum + b2 -> fp32 osb) per m-tile;
    ACT does only tanh; GPSIMD does nothing (measured 0.33 elem/lane/ns).
  - next half-chunk's adds/tanh are emitted interleaved into the current
    m-loop so the PE never waits on ht (v2 lost ~23us to such gaps).
  - output DMAs alternate sync/scalar queues (v2 had a 15us drain tail).
"""

import numpy as np
import ml_dtypes
from contextlib import ExitStack

import concourse.bass as bass
from concourse.bacc import Bacc
import concourse.mybir as mybir
import concourse.tile as tile

B, T, U = 4, 256, 64
D, H, V = 640, 640, 1024
NCORES = 8
TSH = T // NCORES          # 32 T rows per core
BT = B * TSH               # 128 (b, t) rows per core
BU = B * U                 # 256 (b, u) rows
GRID = BT * U              # 8192 grid points per core
P = 128
KD = D // P                # 5 contraction tiles for the input dim
KH = H // P                # 5 contraction tiles for the hidden dim
THALF = 16                 # t rows per half-chunk
HALF = THALF * U           # 1024 grid cols per half-chunk
MHALF = HALF // P          # 8 m-tiles per half-chunk
NHALVES = 2 * B            # 8 half-chunks
F32 = mybir.dt.float32
BF16 = mybir.dt.bfloat16
NPBF = ml_dtypes.bfloat16


def _build():
    nc = Bacc()
    # All inputs arrive pre-arranged in SBUF layout: [128 partitions, ...]
    encT = nc.dram_tensor("encT", [P, KD, BT], BF16, kind="ExternalInput")
    decT = nc.dram_tensor("decT", [P, KD, BU], BF16, kind="ExternalInput")
    # w1 fused per-m: [p, m, half, kd, c] so one 320KB DMA with 2.5KB
    # partition lines covers proj iteration m (input phase is DMA-line-bound).
    w1 = nc.dram_tensor("w1", [P, KH, 2, KD, P], BF16, kind="ExternalInput")
    b1 = nc.dram_tensor("b1", [P, KH], F32, kind="ExternalInput")
    w2 = nc.dram_tensor("w2", [P, KH, V], BF16, kind="ExternalInput")
    b2 = nc.dram_tensor("b2", [V], F32, kind="ExternalInput")
    out = nc.dram_tensor("out", [GRID, V], BF16, kind="ExternalOutput")

    with tile.TileContext(nc) as tc, ExitStack() as ctx:
        const = ctx.enter_context(tc.tile_pool(name="const", bufs=1))
        ht_pool = ctx.enter_context(tc.tile_pool(name="ht", bufs=3))
        osb_pool = ctx.enter_context(tc.tile_pool(name="osb", bufs=4))
        psum = ctx.enter_context(tc.tile_pool(name="psum", bufs=8, space="PSUM"))

        # --- input DMAs: proj-critical slabs first on all 3 queues; w2 and
        # the line-expensive b2 broadcast strictly after (the input phase is
        # DMA-line-bound, ~12us when everything competes at once).
        encT_sb = const.tile([P, KD, BT], BF16, tag="encT")
        w1_sb = const.tile([P, KH, 2, KD, P], BF16, tag="w1")
        b1_sb = const.tile([P, KH], F32, tag="b1")
        decT_sb = const.tile([P, KD, BU], BF16, tag="decT")
        w2_sb = const.tile([P, KH, V], BF16, tag="w2")
        b2_sb = const.tile([P, V], F32, tag="b2")
        # Queue arbitration is line-size-weighted round-robin at ~358GB/s
        # aggregate, so: equal 2.5KB lines on all 3 queues, proj-critical
        # slabs at every head, w2 just-in-time behind, line-expensive
        # broadcasts (b2) last.
        nc.sync.dma_start(w1_sb[:, 0], w1[:][:, 0])
        nc.gpsimd.dma_start(decT_sb[:], decT[:])
        nc.scalar.dma_start(encT_sb[:], encT[:])
        nc.scalar.dma_start(b1_sb[:], b1[:])
        nc.scalar.dma_start(w1_sb[:, 1], w1[:][:, 1])
        nc.gpsimd.dma_start(w1_sb[:, 2], w1[:][:, 2])
        nc.sync.dma_start(w1_sb[:, 3], w1[:][:, 3])
        nc.scalar.dma_start(w1_sb[:, 4], w1[:][:, 4])
        nc.sync.dma_start(w2_sb[:, 0, :], w2[:][:, 0, :])
        nc.gpsimd.dma_start(w2_sb[:, 1, :], w2[:][:, 1, :])
        nc.scalar.dma_start(w2_sb[:, 2, :], w2[:][:, 2, :])
        nc.sync.dma_start(w2_sb[:, 3, :], w2[:][:, 3, :])
        nc.gpsimd.dma_start(w2_sb[:, 4, :], w2[:][:, 4, :])
        nc.gpsimd.dma_start(b2_sb[:], b2[:][None, :].to_broadcast((P, V)))

        # --- PE warm-up: burn the HAM clock-gate ramp during the ~6us engine
        # preamble + input-DMA window; the initial block ends as the first
        # proj slabs land (~10.8us). Extra blocks between proj iterations
        # bridge DMA waits so the PE never looks idle to the clock gate.
        warm_sb = const.tile([P, P], BF16, tag="warm")
        nc.vector.memset(warm_sb[:], 0.0)

        def warm_block(n):
            wps = psum.tile([P, 512], F32, tag="mm", name="wps")
            for _ in range(n):
                nc.tensor.matmul(wps[:, :64], lhsT=warm_sb[:],
                                 rhs=warm_sb[:, :64], start=True, stop=True)

        warm_block(88)

        # --- projections (bf16 out): epb = w1e.T@encT + b1, dp = w1d.T@decT
        epb = const.tile([P, KH, BT], BF16, tag="epb")
        dp = const.tile([P, KH, BU], BF16, tag="dp")
        for m in range(KH):
            pt = psum.tile([P, 512], F32, tag="mm", name="pt")
            for kd in range(KD):
                nc.tensor.matmul(
                    pt[:, :BT],
                    lhsT=w1_sb[:, m, 0, kd, :],
                    rhs=encT_sb[:, kd, :],
                    start=(kd == 0), stop=(kd == KD - 1),
                )
            nc.vector.tensor_scalar_add(epb[:, m, :], pt[:, :BT], b1_sb[:, m:m + 1])
            pt2 = psum.tile([P, 512], F32, tag="mm", name="pt2")
            for kd in range(KD):
                nc.tensor.matmul(
                    pt2[:, :BU],
                    lhsT=w1_sb[:, m, 1, kd, :],
                    rhs=decT_sb[:, kd, :],
                    start=(kd == 0), stop=(kd == KD - 1),
                )
            nc.vector.tensor_copy(dp[:, m, :], pt2[:, :BU])
            if m < KH - 1:
                warm_block((32, 16, 12, 12)[m])

        # --- main grid loop: 8 half-chunks of 1024 cols (16t x 64u) --------
        # ht prep for half-chunk hf; k-granular so deps stay fine-grained
        ht_tiles = {}

        def prep_add(hf, k):
            b, sub = divmod(hf, 2)
            t0 = b * TSH + sub * THALF
            if k == 0:
                ht_tiles[hf] = ht_pool.tile([P, KH, HALF], BF16, tag="ht", name="ht")
            ht = ht_tiles[hf]
            nc.vector.tensor_tensor(
                ht[:, k, :].rearrange("p (t u) -> p t u", u=U),
                epb[:, k, t0:t0 + THALF][:, :, None].to_broadcast((P, THALF, U)),
                dp[:, k, b * U:(b + 1) * U][:, None, :].to_broadcast((P, THALF, U)),
                mybir.AluOpType.add,
            )

        def prep_tanh(hf, k):
            ht = ht_tiles[hf]
            nc.scalar.activation(ht[:, k, :], ht[:, k, :],
                                 mybir.ActivationFunctionType.Tanh)

        for k in range(KH):
            prep_add(0, k)
            prep_tanh(0, k)

        def copyback(hf, mi, pts):
            osb = osb_pool.tile([P, V], BF16, tag="osb")
            for nh in range(2):
                sl = slice(nh * 512, (nh + 1) * 512)
                nc.vector.tensor_tensor(osb[:, sl], pts[nh],
                                        b2_sb[:, sl], mybir.AluOpType.add)
            row0 = (hf * MHALF + mi) * P
            q = (nc.sync, nc.scalar, nc.gpsimd)[mi % 3]
            q.dma_start(out[:][row0:row0 + P, :], osb[:])

        # --- hf=0: two blocks of 4 m-tiles with k as the middle loop (all 8
        # PSUM banks live). Each w2 k-slab is consumed 4x slower than in
        # per-mi order, so the w2 DMAs stream in JIT behind the w1 slabs and
        # the vocab phase can start as soon as proj m4 + ht are done.
        ht0 = ht_tiles.pop(0)
        for blk in range(2):
            mis = list(range(blk * 4, blk * 4 + 4))
            pts = {mi: [psum.tile([P, 512], F32, tag="mm", name="mm")
                        for _ in range(2)] for mi in mis}
            for k in range(KH):
                for mi in mis:
                    for nh in range(2):
                        nc.tensor.matmul(
                            pts[mi][nh],
                            lhsT=ht0[:, k, mi * P:(mi + 1) * P],
                            rhs=w2_sb[:, k, nh * 512:(nh + 1) * 512],
                            start=(k == 0), stop=(k == KH - 1),
                        )
                # weave hf=1 prep between k-sweeps
                if blk == 0 and k >= 1:
                    prep_add(1, k - 1)
                elif blk == 1 and k == 0:
                    prep_add(1, KH - 1)
                elif blk == 1 and 1 <= k <= 2:
                    prep_tanh(1, 2 * k - 2)
                    prep_tanh(1, 2 * k - 1)
                elif blk == 1 and k == 3:
                    prep_tanh(1, KH - 1)
            for mi in mis:
                copyback(0, mi, pts[mi])

        for hf in range(1, NHALVES):
            ht = ht_tiles.pop(hf)
            for mi in range(MHALF):
                # interleave next half's prep into this m-loop
                if hf + 1 < NHALVES:
                    if 1 <= mi <= KH:
                        prep_add(hf + 1, mi - 1)
                    elif mi == KH + 1:
                        for k in range(KH):
                            prep_tanh(hf + 1, k)
                pts = [psum.tile([P, 512], F32, tag="mm", name="mm")
                       for _ in range(2)]
                for k in range(KH):
                    for nh in range(2):
                        nc.tensor.matmul(
                            pts[nh],
                            lhsT=ht[:, k, mi * P:(mi + 1) * P],
                            rhs=w2_sb[:, k, nh * 512:(nh + 1) * 512],
                            start=(k == 0), stop=(k == KH - 1),
                        )
                copyback(hf, mi, pts)

    return nc


_NC_CACHE = {}


def _get_nc(key="bf16v21"):
    if key not in _NC_CACHE:
        nc = _build()
        if not nc.is_finalized():
            nc.finalize()
        _NC_CACHE[key] = nc
    return _NC_CACHE[key]


def make_in_maps(enc_state, dec_state, w1, b1, w2, b2):
    enc_state = np.ascontiguousarray(enc_state, dtype=np.float32)
    dec_flat = np.asarray(dec_state, dtype=np.float32).reshape(BU, D)
    # [d, n] -> [p, kd, n] with d = kd*128 + p
    decT_r = np.ascontiguousarray(
        dec_flat.T.reshape(KD, P, BU).transpose(1, 0, 2)).astype(NPBF)
    # [P, m, half, kd, c]: w1[half*640 + kd*128 + p, m*128 + c]
    w1_r = np.ascontiguousarray(
        np.asarray(w1, dtype=np.float32).reshape(2, KD, P, KH, P)
        .transpose(2, 3, 0, 1, 4)).astype(NPBF)
    b1_r = np.ascontiguousarray(
        np.asarray(b1, dtype=np.float32).reshape(KH, P).T)
    w2_r = np.ascontiguousarray(
        np.asarray(w2, dtype=np.float32).reshape(KH, P, V)
        .transpose(1, 0, 2)).astype(NPBF)
    b2_f = np.ascontiguousarray(b2, dtype=np.float32)
    in_maps = []
    for c in range(NCORES):
        slab = enc_state[:, c * TSH:(c + 1) * TSH, :].reshape(BT, D)
        encT_r = np.ascontiguousarray(
            slab.T.reshape(KD, P, BT).transpose(1, 0, 2)).astype(NPBF)
        in_maps.append({
            "encT": encT_r,
            "decT": decT_r,
            "w1": w1_r,
            "b1": b1_r,
            "w2": w2_r,
            "b2": b2_f,
        })
    return in_maps


def kernel(enc_state, dec_state, w1, b1, w2, b2):
    from concourse.bass_utils import run_bass_kernel_spmd

    nc = _get_nc()
    in_maps = make_in_maps(enc_state, dec_state, w1, b1, w2, b2)
    res = run_bass_kernel_spmd(nc, in_maps, core_ids=list(range(NCORES)))
    shards = [np.asarray(res.results[c]["out"]).astype(np.float32)
              .reshape(B, TSH, U, V) for c in range(NCORES)]
    return np.concatenate(shards, axis=1)


if __name__ == "__main__":
    rng = np.random.default_rng(0)
    ins = {
        "enc_state": rng.standard_normal((B, T, D), dtype=np.float32),
        "dec_state": rng.standard_normal((B, U, D), dtype=np.float32),
        "w1": rng.standard_normal((2 * D, H), dtype=np.float32) / np.sqrt(2 * D),
        "b1": rng.standard_normal((H,), dtype=np.float32) * 0.02,
        "w2": rng.standard_normal((H, V), dtype=np.float32) / np.sqrt(H),
        "b2": rng.standard_normal((V,), dtype=np.float32) * 0.02,
    }
    out = kernel(**ins)
    print(out.shape, out.dtype)

